# revision 25
# baseline (speedup 1.0000x reference)
"""Trainium2 Bass kernel for CustomPositionsPiecewiseConv2d.

Math: for knots positions=[-1,-.5,0,.5,1] and inputs x in [0,1], the
piecewise-linear point interpolation collapses to an exact affine basis
{1, v, min(v,.5)} per patch value v:

    phi_{o,c,k}(v) = W2 + 2*(W4-W3)*v + 2*(2*W3-W2-W4)*min(v,.5)

(W_p = weights[o,c,p,kh,kw]).  Exact everywhere on [0,1] including v=0
(zero-padding -> W2; both plane pads are 0) and v=1 (-> W4, so the
isclose(v,1) mask is absorbed up to ~2e-6).  The constant folds into
the bias: bias_eff = bias + sum_ck W2.  What remains is ONE standard
3x3 conv with 64 input feature channels per image: plane V = v (raw x,
no transform) and plane M = min(v,.5) (one DVE tensor_scalar in place).

Device mapping (per core, 2 images, bf16 data / fp32 PSUM):
  planes [128p, 66, 66] bf16:  [V_im0 | M_im0 | V_im1 | M_im1]
  The host ships x padded, duplicated and pre-cast to bf16 as one
  [128, 66, 66] slab, so the whole input is 3 contiguous chunk DMAs
  and phi is just 2 small DVE min ops per chunk.
  GEMM: per output row-tile (8 rows x 64 cols = 512 px) accumulate 9
  taps in PSUM as K=64 bf16 matmuls.  The two images run on the two
  halves of the PE array via row tiling (tile_position (0,0)/(64,0)
  auto-derived from base partitions), each into its own PSUM banks.

Sharding: data-parallel over batch, 2 images per core on 8 cores.
"""

import numpy as np

B, C, H, W = 16, 32, 64, 64
O, P, KH, KW = 128, 5, 3, 3
NCORES = 8
IPC = B // NCORES            # images per core
HP, WP = H + 2, W + 2        # padded image (pad=1)
RT = 8                       # output rows per L-tile
NT = H // RT                 # L-tiles per image
K2 = KH * KW
ATOL = 1e-5
RTOL = 1e-5
CHUNKS = [(0, 13), (13, 26), (26, 46), (46, 66)]   # DMA/phi row chunks
                                         # (padded rows); chunk 0 -> tile 0,
                                         # chunk 1 -> tiles 1-2


# ---------------------------------------------------------------- host math


def _isclose_np(a, b):
    return np.abs(a - b) <= np.float32(ATOL) + np.float32(RTOL) * np.abs(b)


def _reference_np(x, weights, bias, positions):
    """Direct numpy port of the reference (fallback path)."""
    EPS = 1e-6
    Bn, Cn, Hn, Wn = x.shape
    On, _, Pn, KHn, KWn = weights.shape
    xp = np.pad(x, ((0, 0), (0, 0), (1, 1), (1, 1)))
    cols = [
        xp[:, :, i : i + Hn, j : j + Wn] for i in range(KHn) for j in range(KWn)
    ]
    pat = np.stack(cols, axis=2)
    v = pat.reshape(Bn, Cn, KHn * KWn, Hn * Wn).astype(np.float32)

    left, right = positions[:-1], positions[1:]
    denom = right - left
    denom = np.where(denom == 0, np.float32(EPS), denom)
    varc = (1.0 / denom).astype(np.float32)
    const = (-left * varc).astype(np.float32)

    m_first = _isclose_np(v, positions[0])
    m_last = _isclose_np(v, positions[-1])
    in_range = (~(m_first | m_last)) & (v >= positions[0]) & (v <= positions[-1])

    coeff = np.zeros(v.shape + (Pn,), np.float32)
    coeff[..., 0] += m_first.astype(np.float32)
    coeff[..., Pn - 1] += m_last.astype(np.float32)
    for p in range(Pn - 1):
        m = (in_range & (v >= positions[p]) & (v < positions[p + 1])).astype(
            np.float32
        )
        t = v * varc[p] + const[p]
        coeff[..., p] += m * (1.0 - t)
        coeff[..., p + 1] += m * t

    Wk = np.transpose(weights, (0, 1, 3, 4, 2)).reshape(On, Cn, KHn * KWn, Pn)
    ident = np.all(np.abs(Wk - 1.0) <= np.float32(ATOL + RTOL), axis=-1)
    Wk_eff = np.where(ident[..., None], np.float32(0.0), Wk)

    out = np.einsum("bcklp,ockp->bol", coeff, Wk_eff, optimize=True)
    out = out + np.einsum(
        "bckl,ock->bol", v, ident.astype(np.float32), optimize=True
    )
    out = out + bias[None, :, None]
    return out.reshape(Bn, On, Hn, Wn).astype(np.float32)


def _host_weights(weights, bias):
    """Fold to the {1, v, min(v,.5)} basis.  Returns (wstack [128, K2, O]
    f32 = [Wv; Wm; Wv; Wm] stationary rows, bias_eff [O] f32, ident_any)."""
    Wk = np.transpose(weights, (0, 1, 3, 4, 2)).reshape(O, C, K2, P)
    ident = np.all(np.abs(Wk - 1.0) <= np.float32(ATOL + RTOL), axis=-1)
    ident_any = bool(ident.any())
    Wk_eff = np.where(ident[..., None], np.float32(0.0), Wk).astype(np.float64)
    W2 = Wk_eff[:, :, :, 2]
    W3 = Wk_eff[:, :, :, 3]
    W4 = Wk_eff[:, :, :, 4]
    wv = (2.0 * (W4 - W3)).transpose(1, 2, 0).astype(np.float32)      # [C,K2,O]
    wm = (2.0 * (2.0 * W3 - W2 - W4)).transpose(1, 2, 0).astype(np.float32)
    wstack = np.zeros((4 * C, K2, O), np.float32)
    wstack[0:C] = wv
    wstack[C : 2 * C] = wm
    wstack[2 * C : 3 * C] = wv
    wstack[3 * C : 4 * C] = wm
    bias_eff = (bias.astype(np.float64) + W2.sum(axis=(1, 2))).astype(np.float32)
    return np.ascontiguousarray(wstack), np.ascontiguousarray(bias_eff), ident_any


# ---------------------------------------------------------------- device IR


def _build_nc():
    import concourse.tile as tile
    from concourse import bacc, mybir

    f32 = mybir.dt.float32
    bf16 = mybir.dt.bfloat16
    Alu = mybir.AluOpType
    Act = mybir.ActivationFunctionType

    nc = bacc.Bacc("TRN2", target_bir_lowering=False, debug=False,
                   num_devices=NCORES)
    x_d = nc.dram_tensor("xslab", [4 * C, HP, WP], bf16,
                         kind="ExternalInput").ap()
    w_d = nc.dram_tensor("wstack", [4 * C, K2, O], bf16,
                         kind="ExternalInput").ap()
    b_d = nc.dram_tensor("bias", [O, 1], f32, kind="ExternalInput").ap()
    o_d = nc.dram_tensor("out", [IPC, O, H, W], f32, kind="ExternalOutput").ap()

    with tile.TileContext(nc) as tc:
        with (
            tc.tile_pool(name="const", bufs=1) as constp,
            tc.tile_pool(name="plane", bufs=1) as planep,
            tc.tile_pool(name="psum", bufs=1, space="PSUM") as psump,
            tc.tile_pool(name="osb", bufs=4) as osbp,
        ):
            # ---- planes [V_im0 | M_im0 | V_im1 | M_im1]; V slots are raw x,
            # M slots arrive as raw x and get min(x, 0.5) applied in place.
            # In DMAs ride the scalar HWDGE queue (issued FIRST, before the
            # ACT table load) so chunk 0 lands as early as possible; out DMAs
            # go on sync.
            PL = planep.tile([4 * C, HP, WP], bf16)
            for r0, r1 in CHUNKS:
                nc.scalar.dma_start(PL[:, r0:r1, :], x_d[:, r0:r1, :])

            # ---- weights + bias
            w_sb = constp.tile([4 * C, K2, O], bf16)
            nc.sync.dma_start(w_sb[:], w_d[:])
            b_sb = constp.tile([O, 1], f32)
            nc.sync.dma_start(b_sb[:], b_d[:])

            # ---- ACT table preload (Identity evacs); after the in-DMAs on
            # the scalar queue so it does not delay them
            tiny = constp.tile([C, 1], f32)
            nc.gpsimd.memset(tiny[:], 0.0)
            nc.scalar.activation(tiny[:], tiny[:], Act.Identity, bias=0.0,
                                 scale=1.0)

            # ---- PE warmup: HAM needs ~3.4us of sustained busy to reach
            # K=8/8, and the first-DMA completion latency keeps the planes
            # away until ~4us after the queues open.  Short N=128 dummy
            # matmuls (cheap at the cold clock) keep the PE continuously
            # busy across that whole window so the real stream starts at
            # the warm clock.
            zb = constp.tile([128, 512], bf16)
            nc.gpsimd.memset(zb[:], 0.0)
            for w in range(2):
                pw = psump.tile([O, 128], f32, name=f"ps_warm{w}",
                                tag=f"wa{w}")
                for j in range(20):
                    nc.tensor.matmul(
                        pw[:], zb[0:128, 0:128], zb[:, 0:128],
                        start=(j == 0), stop=(j == 19),
                    )

            for r0, r1 in CHUNKS:
                for i in range(IPC):
                    pm = PL[2 * C * i + C : 2 * C * (i + 1), r0:r1, :]
                    nc.vector.tensor_scalar(pm, pm, 0.5, None, Alu.min)

            # ---- GEMM: per tile, 9 taps x 2 images (row-tiled halves)
            for t in range(NT):
                pss = [
                    psump.tile([O, 512], f32, name=f"ps{i}_{t}",
                               tag=f"ps{i}_{t % 3}")
                    for i in range(IPC)
                ]
                for ki in range(K2):
                    kh, kw = divmod(ki, KW)
                    for i in range(IPC):
                        nc.tensor.matmul(
                            pss[i][:],
                            w_sb[2 * C * i : 2 * C * (i + 1), ki, :],
                            PL[2 * C * i : 2 * C * (i + 1),
                               t * RT + kh : t * RT + kh + RT,
                               kw : kw + W],
                            start=(ki == 0), stop=(ki == K2 - 1),
                        )
                for i in range(IPC):
                    osb = osbp.tile([O, RT * W], f32, name="osb")
                    if t == NT - 1:
                        # final tile: halve the evac across both engines so
                        # the last out-DMA (whose receipt gates teardown)
                        # issues as early as possible
                        hw_ = RT * W // 2
                        hr = RT // 2
                        for h in range(2):
                            sl = slice(h * hw_, (h + 1) * hw_)
                            if (i + h) % 2 == 0:
                                nc.scalar.activation(
                                    osb[:, sl], pss[i][:, sl], Act.Identity,
                                    bias=b_sb[:, 0:1], scale=1.0)
                            else:
                                nc.vector.tensor_scalar(
                                    osb[:, sl], pss[i][:, sl],
                                    b_sb[:, 0:1], None, Alu.add)
                            nc.sync.dma_start(
                                o_d[i, :,
                                    t * RT + h * hr : t * RT + (h + 1) * hr,
                                    :],
                                osb[:, sl].rearrange("o (r w) -> o r w", r=hr),
                            )
                        continue
                    if (t + i) % 2 == 0:
                        nc.scalar.activation(
                            osb[:], pss[i][:], Act.Identity,
                            bias=b_sb[:, 0:1], scale=1.0)
                    else:
                        nc.vector.tensor_scalar(
                            osb[:], pss[i][:], b_sb[:, 0:1], None, Alu.add)
                    nc.sync.dma_start(
                        o_d[i, :, t * RT : (t + 1) * RT, :],
                        osb[:].rearrange("o (r w) -> o r w", r=RT),
                    )
    nc.compile()
    return nc


# ---------------------------------------------------------------- entry


def _prep(inputs):
    x = np.ascontiguousarray(np.asarray(inputs["x"], dtype=np.float32))
    weights = np.ascontiguousarray(np.asarray(inputs["weights"], dtype=np.float32))
    bias = np.ascontiguousarray(np.asarray(inputs["bias"], dtype=np.float32))
    positions = np.ascontiguousarray(
        np.asarray(inputs["positions"], dtype=np.float32)
    )
    return x, weights, bias, positions


def _fast_path_ok(x, positions):
    expect = np.linspace(-1.0, 1.0, P, dtype=np.float32)
    return (
        x.shape == (B, C, H, W)
        and positions.shape == (P,)
        and np.array_equal(positions, expect)
        and float(x.min()) >= 0.0
        and float(x.max()) <= 1.0
    )


def _host_slabs(x):
    """Per-core [4C, HP, WP] bf16 slabs: [x_im0; x_im0; x_im1; x_im1]."""
    import ml_dtypes

    xpad = np.pad(x, ((0, 0), (0, 0), (1, 1), (1, 1))).astype(ml_dtypes.bfloat16)
    slabs = []
    for core in range(NCORES):
        ims = [xpad[core * IPC + i] for i in range(IPC)]
        slab = np.concatenate([ims[0], ims[0], ims[1], ims[1]], axis=0)
        slabs.append(np.ascontiguousarray(slab))
    return slabs


def kernel(**inputs):
    x, weights, bias, positions = _prep(inputs)
    if not _fast_path_ok(x, positions):
        return _reference_np(x, weights, bias, positions)

    wstack, bias_eff, ident_any = _host_weights(weights, bias)
    if ident_any:
        # identity-shortcut weights present: needs the raw-v plane; use the
        # exact fallback rather than a rarely-exercised device path
        return _reference_np(x, weights, bias, positions)

    import ml_dtypes
    from concourse.bass_utils import run_bass_kernel_spmd

    nc = _build_nc()
    slabs = _host_slabs(x)
    wst16 = np.ascontiguousarray(wstack.astype(ml_dtypes.bfloat16))
    bias2d = np.ascontiguousarray(bias_eff.reshape(O, 1))
    in_maps = [
        {"xslab": slabs[i], "wstack": wst16, "bias": bias2d}
        for i in range(NCORES)
    ]
    res = run_bass_kernel_spmd(nc, in_maps, core_ids=list(range(NCORES)))
    out = np.concatenate([res.results[i]["out"] for i in range(NCORES)], axis=0)
    return np.ascontiguousarray(out)


# ------------------------------------------------------------ dev utilities


def _run_sim(inputs):
    """CoreSim single-core run (images 0..IPC-1) for correctness debugging."""
    import ml_dtypes
    from concourse.bass_interp import CoreSim

    x, weights, bias, positions = _prep(inputs)
    assert _fast_path_ok(x, positions)
    wstack, bias_eff, ident_any = _host_weights(weights, bias)
    assert not ident_any
    nc = _build_nc()
    sim = CoreSim(nc)
    sim.tensor("xslab")[:] = _host_slabs(x)[0]
    sim.tensor("wstack")[:] = wstack.astype(ml_dtypes.bfloat16)
    sim.tensor("bias")[:] = bias_eff.reshape(O, 1)
    sim.simulate()
    return np.array(sim.tensor("out"))
